# revision 23
# baseline (speedup 1.0000x reference)
"""Trainium2 Bass kernel for nn_DivergenceRN (gnn_message_passing).

Reference computes, per batch b:
    Z_XX[b,i,:] = max_j relu(X[b,j]@W1a_xx + X[b,i]@W1c_xx + b1_xx) @ W_xx2
    Z_YX[b,i,:] = max_j relu(Y[b,j]@W1a_yx + X[b,i]@W1c_yx + b1_yx) @ W_yx2
    Z = sum_i (Z_XX - Z_YX);  out = relu(cat(Z,Z)@Wd1+bd1)@Wd2+bd2
(The YY / XY branches in the reference are dead code — output-independent.)

v3 pipeline ("diff-relu max"), partitions = 64 h-channels x {xx, yx}:
  Phase 1 (once): pa[b] = blockdiag(W1a)^T @ [X^T;Y^T][b] (4 matmuls) ->
     SBUF fp16; pcb[:,u] = [W1c|b1]^T @ [Xi;1] (1 matmul) -> SBUF f32.
  Per unit u=(b,i), exploiting max(h1,h2) = h1 + relu(h2-h1) so the max
  over j=384 needs only ONE 192-wide PSUM stream on the DVE (PSUM has a
  single DVE read port; dual-PSUM tensor ops are illegal):
    1. relu-pre (one op): rp = max(pa[b] + pcb[:,u], 0) -> fp16, spread
       across DVE (tensor_scalar, 4x mode), GPSIMD, and ScalarE.
    2. PE: h1 = W2^T rp[:,0:192] -> bank lo; d = W2^T rp[:,192:384]
       accum (-W2)^T rp[:,0:192] -> bank hi   (d = h2 - h1; 576 cols).
    3. ScalarE: rr = relu(d) -> SBUF fp16, batched over the 2-unit group
       (no per-unit scalar, so the 352-cycle ACT overhead amortizes).
    4. custom DVE op (1 pass): out = h1 + rr; accum_out = max over j
       -> strip[:, u].  192-elem stream, one PSUM + one SBUF operand.
  Final: strip summed over i, DMA out; host does cross-core sum + decoder.
Sharding: i in [0,384) split across 8 cores (48 rows per core per batch).
"""

import numpy as np

import concourse.bacc as bacc
import concourse.mybir as mybir
import concourse.tile as tile
from concourse.bass_utils import run_bass_kernel_spmd


def _register_addmax():
    """Custom DVE op: out = in0 + in1; accum_out = max over free dim."""
    import concourse.dve_ops as dve_ops
    from concourse.dve_ops import OPS, DveOp
    from concourse.dve_spec import MaxNeg, Spec, Src0, Src1, maxx

    name = "ADD_MAX_REDUCE_K"
    for o in OPS:
        if o.name == name:
            return o

    def _ref(in0, in1, c0, c1, c2):
        b = in0.astype(np.float32) + in1.astype(np.float32)
        return b, b.reshape(b.shape[0], -1).max(axis=-1, keepdims=True)

    op = DveOp(
        name,
        Spec(body=Src0 + Src1, accum=maxx, accum_init=MaxNeg, reference=_ref),
        subdim=False,
        uops_sha={"v3": "b901c41156a86946", "v4": "c9dee8c65593bc95"},
    )
    OPS.append(op)
    dve_ops._SUB_OPCODE_FOR_NAME[name] = (
        dve_ops._CUSTOM_DVE_ROW_BASE + len(OPS) - 1
    )
    return op


ADD_MAX = _register_addmax()

B, N, M, D, H = 4, 384, 384, 64, 64
NCORES = 8
NI = N // NCORES          # i-rows per core per batch
UNITS = B * NI            # 192 (b,i) units per core
JH = M // 2               # 192, half of the j range
P = 2 * H                 # 128 partitions: h x {xx, yx}
BLOB_W = B * N + B * NI + 2 * P   # packed input blob columns

F32 = mybir.dt.float32
F32R = mybir.dt.float32r
F16 = mybir.dt.float16
AX = mybir.AxisListType
ALU = mybir.AluOpType
ACTF = mybir.ActivationFunctionType

# relu-pre engine per (u % 8): D=vector, G=gpsimd, A=scalar
RELU_PAT = "DDDADDDA"
G = 4                     # units per group (ACT relu-d batch, PSUM tile)


def build_nc():
    nc = bacc.Bacc("TRN2", target_bir_lowering=False)

    blob = nc.dram_tensor("blob", [P, BLOB_W], F32R, kind="ExternalInput")
    w2pack = nc.dram_tensor("w2pack", [P, 3 * P], F16, kind="ExternalInput")
    out = nc.dram_tensor("out", [P, B], F32, kind="ExternalOutput")

    with tile.TileContext(nc) as tc:
        with (
            tc.tile_pool(name="singles", bufs=1) as singles,
            tc.tile_pool(name="rp", bufs=8) as rp_pool,
            tc.tile_pool(name="rr", bufs=4) as rr_pool,
            tc.tile_pool(name="m1ps", bufs=2, space="PSUM") as m1_pool,
            tc.tile_pool(name="dps", bufs=2, space="PSUM") as d_pool,
        ):
            blob_s = singles.tile([P, BLOB_W], F32R)
            w2_s = singles.tile([P, 3, P], F16)
            pa16 = singles.tile([P, B, N], F16)
            pcb = singles.tile([P, UNITS], F32)
            strip = singles.tile([P, B, NI], F32)
            scrap = singles.tile([P, JH], F16)
            acc = singles.tile([P, B], F32)

            warm = singles.tile([P, 1], F32)
            nc.vector.memset(warm, 0.0)
            nc.scalar.activation(out=warm, in_=warm, func=ACTF.Relu, scale=1.0)

            nc.sync.dma_start(out=blob_s[:, :], in_=blob[:, :])
            nc.sync.dma_start(out=w2_s[:, :, :], in_=w2pack[:, :])
            o = 0
            xyt_s = blob_s[:, o : o + B * N].rearrange("p (b n) -> p b n", b=B)
            o += B * N
            # xitb: [65, UNITS] — rows 0-63 Xi^T, row 64 = 1.0 (bias lane)
            xitb_s = blob_s[0 : D + 1, o : o + B * NI]
            o += B * NI
            w1ad_s = blob_s[:, o : o + P]
            o += P
            # w1cb: [65, 128] — rows 0-63 = [W1c_xx | W1c_yx], row 64 = b1^T
            w1cb_s = blob_s[0 : D + 1, o : o + P]
            o += P
            assert o == BLOB_W

            # Phase 1: pcb (one matmul), pa16 (one matmul + copy per b),
            # using main-loop PSUM tiles (bank regions) to stay in 8 banks.
            ph = [
                m1_pool.tile([P, 2, 512], F32, tag="m", name="phm0"),
                m1_pool.tile([P, 2, 512], F32, tag="m", name="phm1"),
                d_pool.tile([P, 2, 512], F32, tag="d", name="phd0"),
            ]
            nc.tensor.matmul(
                ph[0][:, 0, 0:UNITS], lhsT=w1cb_s, rhs=xitb_s, start=True, stop=True
            )
            nc.vector.tensor_scalar(
                out=pcb[:, :], in0=ph[0][:, 0, 0:UNITS],
                scalar1=0.0, scalar2=None, op0=ALU.add,
            )
            for b in range(B):
                t, r = ph[(b + 1) // 2], (b + 1) % 2
                nc.tensor.matmul(
                    t[:, r, 0:N], lhsT=w1ad_s, rhs=xyt_s[:, b, :],
                    start=True, stop=True,
                )
                nc.scalar.mul(pa16[:, b, :], t[:, r, 0:N], 1.0)

            def relu_pre(u, rp):
                b = u // NI
                kind = RELU_PAT[u % len(RELU_PAT)]
                eng = {"D": nc.vector, "G": nc.gpsimd, "A": None}[kind]
                if eng is None:
                    nc.scalar.activation(
                        out=rp, in_=pa16[:, b, :], func=ACTF.Relu,
                        bias=pcb[:, u : u + 1], scale=1.0,
                    )
                else:
                    eng.tensor_scalar(
                        out=rp, in0=pa16[:, b, :],
                        scalar1=pcb[:, u : u + 1], scalar2=0.0,
                        op0=ALU.add, op1=ALU.max,
                    )

            # Main loop over 4-unit supergroups. PSUM layout per group:
            #   m1-tile [P,2,512]: bank p holds h1(u2p)|h1(u2p+1) (192 each);
            #   d-tile  [P,2,512]: bank p holds d(u2p)|d(u2p+1), d = h2-h1.
            # ScalarE relus the whole d-tile (one op per 4 units); an
            # identity matmul accumulates rr onto h1 so the m1-tile becomes
            # max(h1,h2) pairs; one batched tensor_reduce then maxes all 4
            # units ([P,2,2,192] -> strip[:, b, i:i+4]).
            def mmid(pend):
                m1t, rr, k = pend
                for p in range(2):
                    nc.tensor.matmul(
                        m1t[:, p, 0:M], lhsT=w2_s[:, 2, :], rhs=rr[:, p, :],
                        start=False, stop=True, skip_group_check=True,
                    )

            def reduce(pend):
                m1t, rr, k = pend
                b, i0 = divmod(4 * k, NI)
                nc.vector.tensor_reduce(
                    out=strip[:, b, i0 : i0 + 4],
                    in_=m1t[:, :, 0:M].rearrange("p a (c j) -> p a c j", c=2),
                    axis=AX.X, op=ALU.max,
                )

            pend = None  # (m1_tile, rr, k)
            for k in range(UNITS // 4):
                rps = []
                for g in range(4):
                    rp = rp_pool.tile([P, N], F16)
                    relu_pre(k * 4 + g, rp)
                    rps.append(rp)
                    if g == 1 and pend is not None:
                        mmid(pend)
                        reduce(pend)
                m1t = m1_pool.tile([P, 2, 512], F32, tag="m")
                dt = d_pool.tile([P, 2, 512], F32, tag="d")
                for g in range(4):
                    nc.tensor.matmul(
                        m1t[:, g // 2, (g % 2) * JH : (g % 2) * JH + JH],
                        lhsT=w2_s[:, 0, :], rhs=rps[g][:, 0:JH],
                        start=(g % 2 == 0), stop=False, skip_group_check=True,
                    )
                for g in range(4):
                    nc.tensor.matmul(
                        dt[:, g // 2, (g % 2) * JH : (g % 2) * JH + JH],
                        lhsT=w2_s[:, 0, :], rhs=rps[g][:, JH:M],
                        start=(g % 2 == 0), stop=False, skip_group_check=True,
                    )
                for g in range(4):
                    nc.tensor.matmul(
                        dt[:, g // 2, (g % 2) * JH : (g % 2) * JH + JH],
                        lhsT=w2_s[:, 1, :], rhs=rps[g][:, 0:JH],
                        start=False, stop=True, skip_group_check=True,
                    )
                rr_k = rr_pool.tile([P, 2, M], F16)
                nc.scalar.activation(
                    out=rr_k[:, :, :], in_=dt[:, :, 0:M],
                    func=ACTF.Relu, scale=1.0,
                )
                pend = (m1t, rr_k, k)
            mmid(pend)
            reduce(pend)

            nc.vector.tensor_reduce(
                out=acc[:, :], in_=strip[:, :, :], axis=AX.X, op=ALU.add
            )
            nc.sync.dma_start(out=out[:, :], in_=acc[:, :])

    nc.compile()
    return nc


def _prep_inputs(X, Y, W_xx1, W_yx1, b_xx1, b_yx1, W_xx2, W_yx2):
    """Host-side input prep shared by all cores (except xit)."""
    f = np.float32
    XYT = np.ascontiguousarray(
        np.concatenate([X.transpose(0, 2, 1), Y.transpose(0, 2, 1)], axis=1), f
    )  # [B, 128, N]
    W1ad = np.zeros((P, P), f)
    W1ad[:D, :H] = W_xx1[:D]
    W1ad[D:, H:] = W_yx1[:D]
    W1c = np.ascontiguousarray(np.concatenate([W_xx1[D:], W_yx1[D:]], axis=1), f)
    b1v = np.concatenate([b_xx1, b_yx1]).reshape(P, 1).astype(f)
    W2bd = np.zeros((P, P), f)
    W2bd[:H, :H] = W_xx2
    W2bd[H:, H:] = W_yx2
    return XYT, W1ad, W1c, b1v, W2bd


def _pack_blob(XYT, XiT, W1ad, W1c, b1v):
    """Pack per-core inputs into the [P, BLOB_W] blob (see build_nc)."""
    f = np.float32
    blob = np.zeros((P, BLOB_W), f)
    o = 0
    blob[:, o : o + B * N] = XYT.transpose(1, 0, 2).reshape(P, B * N)
    o += B * N
    blob[:D, o : o + B * NI] = XiT.transpose(1, 0, 2).reshape(D, B * NI)
    blob[D, o : o + B * NI] = 1.0
    o += B * NI
    blob[:, o : o + P] = W1ad
    o += P
    blob[:D, o : o + P] = W1c
    blob[D, o : o + P] = b1v[:, 0]
    o += P
    assert o == BLOB_W
    return blob


def kernel(
    X, Y,
    W_xx1, b_xx1, W_xx2, b_xx2,
    W_xy1, b_xy1, W_xy2, b_xy2,
    W_yx1, b_yx1, W_yx2, b_yx2,
    W_yy1, b_yy1, W_yy2, b_yy2,
    Wd1, bd1, Wd2, bd2,
    _trace=False, _tmpdir=None,
):
    f = np.float32
    X = np.asarray(X, f)
    Y = np.asarray(Y, f)
    XYT, W1ad, W1c, b1v, W2bd = _prep_inputs(
        X, Y, W_xx1, W_yx1, b_xx1, b_yx1, W_xx2, W_yx2
    )
    W2pack = np.concatenate(
        [W2bd, -W2bd, np.eye(P, dtype=np.float32)], axis=1
    ).astype(np.float16)
    W2pack = np.ascontiguousarray(W2pack)

    in_maps = []
    for c in range(NCORES):
        XiT = np.ascontiguousarray(
            X[:, c * NI : (c + 1) * NI, :].transpose(0, 2, 1), f
        )  # [B, 64, NI]
        in_maps.append(
            {
                "blob": _pack_blob(XYT, XiT, W1ad, W1c, b1v),
                "w2pack": W2pack,
            }
        )

    nc = build_nc()
    res = run_bass_kernel_spmd(
        nc,
        in_maps,
        core_ids=list(range(NCORES)),
        trace=_trace,
        tmpdir=_tmpdir,
    )
    acc = np.zeros((P, B), np.float64)
    for r in res.results:
        acc += r["out"].astype(np.float64)
    acc = acc.astype(f)

    # acc[k, b] = sum_i max_j (relu_pre @ W2)[k]  for xx (k<64) / yx (k>=64)
    Zdiff = (acc[:H] - acc[H:]).T + N * (b_xx2 - b_yx2)[None, :]  # [B, H]
    z = np.concatenate([Zdiff, Zdiff], axis=1).astype(f)  # [B, 2H]
    h = np.maximum(z @ Wd1 + bd1, 0.0).astype(f)
    outv = (h @ Wd2 + bd2).astype(f)
    if _trace:
        return outv, res
    return outv


# revision 24
# speedup vs baseline: 1.0383x; 1.0383x over previous
"""Trainium2 Bass kernel for nn_DivergenceRN (gnn_message_passing).

Reference computes, per batch b:
    Z_XX[b,i,:] = max_j relu(X[b,j]@W1a_xx + X[b,i]@W1c_xx + b1_xx) @ W_xx2
    Z_YX[b,i,:] = max_j relu(Y[b,j]@W1a_yx + X[b,i]@W1c_yx + b1_yx) @ W_yx2
    Z = sum_i (Z_XX - Z_YX);  out = relu(cat(Z,Z)@Wd1+bd1)@Wd2+bd2
(The YY / XY branches in the reference are dead code — output-independent.)

v3 pipeline ("diff-relu max"), partitions = 64 h-channels x {xx, yx}:
  Phase 1 (once): pa[b] = blockdiag(W1a)^T @ [X^T;Y^T][b] (4 matmuls) ->
     SBUF fp16; pcb[:,u] = [W1c|b1]^T @ [Xi;1] (1 matmul) -> SBUF f32.
  Per unit u=(b,i), exploiting max(h1,h2) = h1 + relu(h2-h1) so the max
  over j=384 needs only ONE 192-wide PSUM stream on the DVE (PSUM has a
  single DVE read port; dual-PSUM tensor ops are illegal):
    1. relu-pre (one op): rp = max(pa[b] + pcb[:,u], 0) -> fp16, spread
       across DVE (tensor_scalar, 4x mode), GPSIMD, and ScalarE.
    2. PE: h1 = W2^T rp[:,0:192] -> bank lo; d = W2^T rp[:,192:384]
       accum (-W2)^T rp[:,0:192] -> bank hi   (d = h2 - h1; 576 cols).
    3. ScalarE: rr = relu(d) -> SBUF fp16, batched over the 2-unit group
       (no per-unit scalar, so the 352-cycle ACT overhead amortizes).
    4. custom DVE op (1 pass): out = h1 + rr; accum_out = max over j
       -> strip[:, u].  192-elem stream, one PSUM + one SBUF operand.
  Final: strip summed over i, DMA out; host does cross-core sum + decoder.
Sharding: i in [0,384) split across 8 cores (48 rows per core per batch).
"""

import numpy as np

import concourse.bacc as bacc
import concourse.mybir as mybir
import concourse.tile as tile
from concourse.bass_utils import run_bass_kernel_spmd


def _register_addmax():
    """Custom DVE op: out = in0 + in1; accum_out = max over free dim."""
    import concourse.dve_ops as dve_ops
    from concourse.dve_ops import OPS, DveOp
    from concourse.dve_spec import MaxNeg, Spec, Src0, Src1, maxx

    name = "ADD_MAX_REDUCE_K"
    for o in OPS:
        if o.name == name:
            return o

    def _ref(in0, in1, c0, c1, c2):
        b = in0.astype(np.float32) + in1.astype(np.float32)
        return b, b.reshape(b.shape[0], -1).max(axis=-1, keepdims=True)

    op = DveOp(
        name,
        Spec(body=Src0 + Src1, accum=maxx, accum_init=MaxNeg, reference=_ref),
        subdim=False,
        uops_sha={"v3": "b901c41156a86946", "v4": "c9dee8c65593bc95"},
    )
    OPS.append(op)
    dve_ops._SUB_OPCODE_FOR_NAME[name] = (
        dve_ops._CUSTOM_DVE_ROW_BASE + len(OPS) - 1
    )
    return op


ADD_MAX = _register_addmax()

B, N, M, D, H = 4, 384, 384, 64, 64
NCORES = 8
NI = N // NCORES          # i-rows per core per batch
UNITS = B * NI            # 192 (b,i) units per core
JH = M // 2               # 192, half of the j range
P = 2 * H                 # 128 partitions: h x {xx, yx}
BLOB_W = B * N + B * NI + 2 * P   # packed input blob columns

F32 = mybir.dt.float32
F32R = mybir.dt.float32r
F16 = mybir.dt.float16
AX = mybir.AxisListType
ALU = mybir.AluOpType
ACTF = mybir.ActivationFunctionType

# relu-pre engine per (u % 8): D=vector, G=gpsimd, A=scalar
RELU_PAT = "DADDDADD"
G = 4                     # units per group (ACT relu-d batch, PSUM tile)


def build_nc():
    nc = bacc.Bacc("TRN2", target_bir_lowering=False)

    blob = nc.dram_tensor("blob", [P, BLOB_W], F32R, kind="ExternalInput")
    w2pack = nc.dram_tensor("w2pack", [P, 3 * P], F16, kind="ExternalInput")
    out = nc.dram_tensor("out", [P, B], F32, kind="ExternalOutput")

    with tile.TileContext(nc) as tc:
        with (
            tc.tile_pool(name="singles", bufs=1) as singles,
            tc.tile_pool(name="rp", bufs=8) as rp_pool,
            tc.tile_pool(name="rr", bufs=3) as rr_pool,
            tc.tile_pool(name="m1ps", bufs=2, space="PSUM") as m1_pool,
            tc.tile_pool(name="dps", bufs=2, space="PSUM") as d_pool,
        ):
            blob_s = singles.tile([P, BLOB_W], F32R)
            w2_s = singles.tile([P, 3, P], F16)
            pa16 = singles.tile([P, B, N], F16)
            pcb = singles.tile([P, UNITS], F32)
            strip = singles.tile([P, B, NI], F32)
            scrap = singles.tile([P, JH], F16)
            acc = singles.tile([P, B], F32)

            warm = singles.tile([P, 1], F32)
            nc.vector.memset(warm, 0.0)
            nc.scalar.activation(out=warm, in_=warm, func=ACTF.Relu, scale=1.0)

            nc.sync.dma_start(out=blob_s[:, :], in_=blob[:, :])
            nc.sync.dma_start(out=w2_s[:, :, :], in_=w2pack[:, :])
            o = 0
            xyt_s = blob_s[:, o : o + B * N].rearrange("p (b n) -> p b n", b=B)
            o += B * N
            # xitb: [65, UNITS] — rows 0-63 Xi^T, row 64 = 1.0 (bias lane)
            xitb_s = blob_s[0 : D + 1, o : o + B * NI]
            o += B * NI
            w1ad_s = blob_s[:, o : o + P]
            o += P
            # w1cb: [65, 128] — rows 0-63 = [W1c_xx | W1c_yx], row 64 = b1^T
            w1cb_s = blob_s[0 : D + 1, o : o + P]
            o += P
            assert o == BLOB_W

            # Phase 1: pcb (one matmul), pa16 (one matmul + copy per b),
            # using main-loop PSUM tiles (bank regions) to stay in 8 banks.
            ph = [
                m1_pool.tile([P, 2, 512], F32, tag="m", name="phm0"),
                m1_pool.tile([P, 2, 512], F32, tag="m", name="phm1"),
                d_pool.tile([P, 2, 512], F32, tag="d", name="phd0"),
            ]
            nc.tensor.matmul(
                ph[0][:, 0, 0:UNITS], lhsT=w1cb_s, rhs=xitb_s, start=True, stop=True
            )
            nc.vector.tensor_scalar(
                out=pcb[:, :], in0=ph[0][:, 0, 0:UNITS],
                scalar1=0.0, scalar2=None, op0=ALU.add,
            )
            for b in range(B):
                t, r = ph[(b + 1) // 2], (b + 1) % 2
                nc.tensor.matmul(
                    t[:, r, 0:N], lhsT=w1ad_s, rhs=xyt_s[:, b, :],
                    start=True, stop=True,
                )
                nc.scalar.mul(pa16[:, b, :], t[:, r, 0:N], 1.0)

            def relu_pre(u, rp):
                b = u // NI
                kind = RELU_PAT[u % len(RELU_PAT)]
                eng = {"D": nc.vector, "G": nc.gpsimd, "A": None}[kind]
                if eng is None:
                    nc.scalar.activation(
                        out=rp, in_=pa16[:, b, :], func=ACTF.Relu,
                        bias=pcb[:, u : u + 1], scale=1.0,
                    )
                else:
                    eng.tensor_scalar(
                        out=rp, in0=pa16[:, b, :],
                        scalar1=pcb[:, u : u + 1], scalar2=0.0,
                        op0=ALU.add, op1=ALU.max,
                    )

            # Main loop over 4-unit supergroups. PSUM layout per group:
            #   m1-tile [P,2,512]: bank p holds h1(u2p)|h1(u2p+1) (192 each);
            #   d-tile  [P,2,512]: bank p holds d(u2p)|d(u2p+1), d = h2-h1.
            # ScalarE relus the whole d-tile (one op per 4 units); an
            # identity matmul accumulates rr onto h1 so the m1-tile becomes
            # max(h1,h2) pairs; one batched tensor_reduce then maxes all 4
            # units ([P,2,2,192] -> strip[:, b, i:i+4]).
            def mmid(pend):
                m1t, rr, k = pend
                for p in range(2):
                    nc.tensor.matmul(
                        m1t[:, p, 0:M], lhsT=w2_s[:, 2, :], rhs=rr[:, p, :],
                        start=False, stop=True, skip_group_check=True,
                    )

            def reduce(pend):
                m1t, rr, k = pend
                b, i0 = divmod(4 * k, NI)
                nc.vector.tensor_reduce(
                    out=strip[:, b, i0 : i0 + 4],
                    in_=m1t[:, :, 0:M].rearrange("p a (c j) -> p a c j", c=2),
                    axis=AX.X, op=ALU.max,
                )

            pend = None  # (m1_tile, rr, k)
            for k in range(UNITS // 4):
                rps = []
                for g in range(4):
                    rp = rp_pool.tile([P, N], F16)
                    relu_pre(k * 4 + g, rp)
                    rps.append(rp)
                if pend is not None:
                    mmid(pend)
                m1t = m1_pool.tile([P, 2, 512], F32, tag="m")
                dt = d_pool.tile([P, 2, 512], F32, tag="d")
                for g in range(4):
                    nc.tensor.matmul(
                        m1t[:, g // 2, (g % 2) * JH : (g % 2) * JH + JH],
                        lhsT=w2_s[:, 0, :], rhs=rps[g][:, 0:JH],
                        start=(g % 2 == 0), stop=False, skip_group_check=True,
                    )
                for g in range(4):
                    nc.tensor.matmul(
                        dt[:, g // 2, (g % 2) * JH : (g % 2) * JH + JH],
                        lhsT=w2_s[:, 0, :], rhs=rps[g][:, JH:M],
                        start=(g % 2 == 0), stop=False, skip_group_check=True,
                    )
                for g in range(4):
                    nc.tensor.matmul(
                        dt[:, g // 2, (g % 2) * JH : (g % 2) * JH + JH],
                        lhsT=w2_s[:, 1, :], rhs=rps[g][:, 0:JH],
                        start=False, stop=True, skip_group_check=True,
                    )
                rr_k = rr_pool.tile([P, 2, M], F16)
                nc.scalar.activation(
                    out=rr_k[:, :, :], in_=dt[:, :, 0:M],
                    func=ACTF.Relu, scale=1.0,
                )
                if pend is not None:
                    reduce(pend)
                pend = (m1t, rr_k, k)
            mmid(pend)
            reduce(pend)

            nc.vector.tensor_reduce(
                out=acc[:, :], in_=strip[:, :, :], axis=AX.X, op=ALU.add
            )
            nc.sync.dma_start(out=out[:, :], in_=acc[:, :])

    nc.compile()
    return nc


def _prep_inputs(X, Y, W_xx1, W_yx1, b_xx1, b_yx1, W_xx2, W_yx2):
    """Host-side input prep shared by all cores (except xit)."""
    f = np.float32
    XYT = np.ascontiguousarray(
        np.concatenate([X.transpose(0, 2, 1), Y.transpose(0, 2, 1)], axis=1), f
    )  # [B, 128, N]
    W1ad = np.zeros((P, P), f)
    W1ad[:D, :H] = W_xx1[:D]
    W1ad[D:, H:] = W_yx1[:D]
    W1c = np.ascontiguousarray(np.concatenate([W_xx1[D:], W_yx1[D:]], axis=1), f)
    b1v = np.concatenate([b_xx1, b_yx1]).reshape(P, 1).astype(f)
    W2bd = np.zeros((P, P), f)
    W2bd[:H, :H] = W_xx2
    W2bd[H:, H:] = W_yx2
    return XYT, W1ad, W1c, b1v, W2bd


def _pack_blob(XYT, XiT, W1ad, W1c, b1v):
    """Pack per-core inputs into the [P, BLOB_W] blob (see build_nc)."""
    f = np.float32
    blob = np.zeros((P, BLOB_W), f)
    o = 0
    blob[:, o : o + B * N] = XYT.transpose(1, 0, 2).reshape(P, B * N)
    o += B * N
    blob[:D, o : o + B * NI] = XiT.transpose(1, 0, 2).reshape(D, B * NI)
    blob[D, o : o + B * NI] = 1.0
    o += B * NI
    blob[:, o : o + P] = W1ad
    o += P
    blob[:D, o : o + P] = W1c
    blob[D, o : o + P] = b1v[:, 0]
    o += P
    assert o == BLOB_W
    return blob


def kernel(
    X, Y,
    W_xx1, b_xx1, W_xx2, b_xx2,
    W_xy1, b_xy1, W_xy2, b_xy2,
    W_yx1, b_yx1, W_yx2, b_yx2,
    W_yy1, b_yy1, W_yy2, b_yy2,
    Wd1, bd1, Wd2, bd2,
    _trace=False, _tmpdir=None,
):
    f = np.float32
    X = np.asarray(X, f)
    Y = np.asarray(Y, f)
    XYT, W1ad, W1c, b1v, W2bd = _prep_inputs(
        X, Y, W_xx1, W_yx1, b_xx1, b_yx1, W_xx2, W_yx2
    )
    W2pack = np.concatenate(
        [W2bd, -W2bd, np.eye(P, dtype=np.float32)], axis=1
    ).astype(np.float16)
    W2pack = np.ascontiguousarray(W2pack)

    in_maps = []
    for c in range(NCORES):
        XiT = np.ascontiguousarray(
            X[:, c * NI : (c + 1) * NI, :].transpose(0, 2, 1), f
        )  # [B, 64, NI]
        in_maps.append(
            {
                "blob": _pack_blob(XYT, XiT, W1ad, W1c, b1v),
                "w2pack": W2pack,
            }
        )

    nc = build_nc()
    res = run_bass_kernel_spmd(
        nc,
        in_maps,
        core_ids=list(range(NCORES)),
        trace=_trace,
        tmpdir=_tmpdir,
    )
    acc = np.zeros((P, B), np.float64)
    for r in res.results:
        acc += r["out"].astype(np.float64)
    acc = acc.astype(f)

    # acc[k, b] = sum_i max_j (relu_pre @ W2)[k]  for xx (k<64) / yx (k>=64)
    Zdiff = (acc[:H] - acc[H:]).T + N * (b_xx2 - b_yx2)[None, :]  # [B, H]
    z = np.concatenate([Zdiff, Zdiff], axis=1).astype(f)  # [B, 2H]
    h = np.maximum(z @ Wd1 + bd1, 0.0).astype(f)
    outv = (h @ Wd2 + bd2).astype(f)
    if _trace:
        return outv, res
    return outv


# revision 25
# speedup vs baseline: 1.0504x; 1.0117x over previous
"""Trainium2 Bass kernel for nn_DivergenceRN (gnn_message_passing).

Reference computes, per batch b:
    Z_XX[b,i,:] = max_j relu(X[b,j]@W1a_xx + X[b,i]@W1c_xx + b1_xx) @ W_xx2
    Z_YX[b,i,:] = max_j relu(Y[b,j]@W1a_yx + X[b,i]@W1c_yx + b1_yx) @ W_yx2
    Z = sum_i (Z_XX - Z_YX);  out = relu(cat(Z,Z)@Wd1+bd1)@Wd2+bd2
(The YY / XY branches in the reference are dead code — output-independent.)

v3 pipeline ("diff-relu max"), partitions = 64 h-channels x {xx, yx}:
  Phase 1 (once): pa[b] = blockdiag(W1a)^T @ [X^T;Y^T][b] (4 matmuls) ->
     SBUF fp16; pcb[:,u] = [W1c|b1]^T @ [Xi;1] (1 matmul) -> SBUF f32.
  Per unit u=(b,i), exploiting max(h1,h2) = h1 + relu(h2-h1) so the max
  over j=384 needs only ONE 192-wide PSUM stream on the DVE (PSUM has a
  single DVE read port; dual-PSUM tensor ops are illegal):
    1. relu-pre (one op): rp = max(pa[b] + pcb[:,u], 0) -> fp16, spread
       across DVE (tensor_scalar, 4x mode), GPSIMD, and ScalarE.
    2. PE: h1 = W2^T rp[:,0:192] -> bank lo; d = W2^T rp[:,192:384]
       accum (-W2)^T rp[:,0:192] -> bank hi   (d = h2 - h1; 576 cols).
    3. ScalarE: rr = relu(d) -> SBUF fp16, batched over the 2-unit group
       (no per-unit scalar, so the 352-cycle ACT overhead amortizes).
    4. custom DVE op (1 pass): out = h1 + rr; accum_out = max over j
       -> strip[:, u].  192-elem stream, one PSUM + one SBUF operand.
  Final: strip summed over i, DMA out; host does cross-core sum + decoder.
Sharding: i in [0,384) split across 8 cores (48 rows per core per batch).
"""

import numpy as np

import concourse.bacc as bacc
import concourse.mybir as mybir
import concourse.tile as tile
from concourse.bass_utils import run_bass_kernel_spmd


def _register_addmax():
    """Custom DVE op: out = in0 + in1; accum_out = max over free dim."""
    import concourse.dve_ops as dve_ops
    from concourse.dve_ops import OPS, DveOp
    from concourse.dve_spec import MaxNeg, Spec, Src0, Src1, maxx

    name = "ADD_MAX_REDUCE_K"
    for o in OPS:
        if o.name == name:
            return o

    def _ref(in0, in1, c0, c1, c2):
        b = in0.astype(np.float32) + in1.astype(np.float32)
        return b, b.reshape(b.shape[0], -1).max(axis=-1, keepdims=True)

    op = DveOp(
        name,
        Spec(body=Src0 + Src1, accum=maxx, accum_init=MaxNeg, reference=_ref),
        subdim=False,
        uops_sha={"v3": "b901c41156a86946", "v4": "c9dee8c65593bc95"},
    )
    OPS.append(op)
    dve_ops._SUB_OPCODE_FOR_NAME[name] = (
        dve_ops._CUSTOM_DVE_ROW_BASE + len(OPS) - 1
    )
    return op


ADD_MAX = _register_addmax()

B, N, M, D, H = 4, 384, 384, 64, 64
NCORES = 8
NI = N // NCORES          # i-rows per core per batch
UNITS = B * NI            # 192 (b,i) units per core
JH = M // 2               # 192, half of the j range
P = 2 * H                 # 128 partitions: h x {xx, yx}
BLOB_W = B * N + B * NI + 2 * P   # packed input blob columns

F32 = mybir.dt.float32
F32R = mybir.dt.float32r
F16 = mybir.dt.float16
AX = mybir.AxisListType
ALU = mybir.AluOpType
ACTF = mybir.ActivationFunctionType

# relu-pre engine per (u % 8): D=vector, G=gpsimd, A=scalar
RELU_PAT = "DADDDADD"
G = 4                     # units per group (ACT relu-d batch, PSUM tile)


def build_nc():
    nc = bacc.Bacc("TRN2", target_bir_lowering=False)

    blob = nc.dram_tensor("blob", [P, BLOB_W], F32R, kind="ExternalInput")
    w2pack = nc.dram_tensor("w2pack", [P, 3 * P], F16, kind="ExternalInput")
    out = nc.dram_tensor("out", [P, B], F32, kind="ExternalOutput")

    with tile.TileContext(nc) as tc:
        with (
            tc.tile_pool(name="singles", bufs=1) as singles,
            tc.tile_pool(name="rp", bufs=12) as rp_pool,
            tc.tile_pool(name="rr", bufs=3) as rr_pool,
            tc.tile_pool(name="m1ps", bufs=2, space="PSUM") as m1_pool,
            tc.tile_pool(name="dps", bufs=2, space="PSUM") as d_pool,
        ):
            blob_s = singles.tile([P, BLOB_W], F32R)
            w2_s = singles.tile([P, 3, P], F16)
            pa16 = singles.tile([P, B, N], F16)
            pcb = singles.tile([P, UNITS], F32)
            strip = singles.tile([P, B, NI], F32)
            scrap = singles.tile([P, JH], F16)
            acc = singles.tile([P, B], F32)

            warm = singles.tile([P, 1], F32)
            nc.vector.memset(warm, 0.0)
            nc.scalar.activation(out=warm, in_=warm, func=ACTF.Relu, scale=1.0)

            nc.sync.dma_start(out=blob_s[:, :], in_=blob[:, :])
            nc.sync.dma_start(out=w2_s[:, :, :], in_=w2pack[:, :])
            o = 0
            xyt_s = blob_s[:, o : o + B * N].rearrange("p (b n) -> p b n", b=B)
            o += B * N
            # xitb: [65, UNITS] — rows 0-63 Xi^T, row 64 = 1.0 (bias lane)
            xitb_s = blob_s[0 : D + 1, o : o + B * NI]
            o += B * NI
            w1ad_s = blob_s[:, o : o + P]
            o += P
            # w1cb: [65, 128] — rows 0-63 = [W1c_xx | W1c_yx], row 64 = b1^T
            w1cb_s = blob_s[0 : D + 1, o : o + P]
            o += P
            assert o == BLOB_W

            # Phase 1: pcb (one matmul), pa16 (one matmul + copy per b),
            # using main-loop PSUM tiles (bank regions) to stay in 8 banks.
            ph = [
                m1_pool.tile([P, 2, 512], F32, tag="m", name="phm0"),
                m1_pool.tile([P, 2, 512], F32, tag="m", name="phm1"),
                d_pool.tile([P, 2, 512], F32, tag="d", name="phd0"),
            ]
            nc.tensor.matmul(
                ph[0][:, 0, 0:UNITS], lhsT=w1cb_s, rhs=xitb_s, start=True, stop=True
            )
            nc.vector.tensor_scalar(
                out=pcb[:, :], in0=ph[0][:, 0, 0:UNITS],
                scalar1=0.0, scalar2=None, op0=ALU.add,
            )
            for b in range(B):
                t, r = ph[(b + 1) // 2], (b + 1) % 2
                nc.tensor.matmul(
                    t[:, r, 0:N], lhsT=w1ad_s, rhs=xyt_s[:, b, :],
                    start=True, stop=True,
                )
                nc.scalar.mul(pa16[:, b, :], t[:, r, 0:N], 1.0)

            def relu_pre(u, rp):
                b = u // NI
                kind = RELU_PAT[u % len(RELU_PAT)]
                eng = {"D": nc.vector, "G": nc.gpsimd, "A": None}[kind]
                if eng is None:
                    nc.scalar.activation(
                        out=rp, in_=pa16[:, b, :], func=ACTF.Relu,
                        bias=pcb[:, u : u + 1], scale=1.0,
                    )
                else:
                    eng.tensor_scalar(
                        out=rp, in0=pa16[:, b, :],
                        scalar1=pcb[:, u : u + 1], scalar2=0.0,
                        op0=ALU.add, op1=ALU.max,
                    )

            # Main loop over 4-unit supergroups. PSUM layout per group:
            #   m1-tile [P,2,512]: bank p holds h1(u2p)|h1(u2p+1) (192 each);
            #   d-tile  [P,2,512]: bank p holds d(u2p)|d(u2p+1), d = h2-h1.
            # ScalarE relus the whole d-tile (one op per 4 units); an
            # identity matmul accumulates rr onto h1 so the m1-tile becomes
            # max(h1,h2) pairs; one batched tensor_reduce then maxes all 4
            # units ([P,2,2,192] -> strip[:, b, i:i+4]).
            def mmid(pend):
                m1t, rr, k = pend
                for p in range(2):
                    nc.tensor.matmul(
                        m1t[:, p, 0:M], lhsT=w2_s[:, 2, :], rhs=rr[:, p, :],
                        start=False, stop=True, skip_group_check=True,
                    )

            def reduce(pend):
                m1t, rr, k = pend
                b, i0 = divmod(4 * k, NI)
                nc.vector.tensor_reduce(
                    out=strip[:, b, i0 : i0 + 4],
                    in_=m1t[:, :, 0:M].rearrange("p a (c j) -> p a c j", c=2),
                    axis=AX.X, op=ALU.max,
                )

            pend = None  # (m1_tile, rr, k)
            for k in range(UNITS // 4):
                rps = []
                for g in range(4):
                    rp = rp_pool.tile([P, N], F16)
                    relu_pre(k * 4 + g, rp)
                    rps.append(rp)
                if pend is not None:
                    mmid(pend)
                m1t = m1_pool.tile([P, 2, 512], F32, tag="m")
                dt = d_pool.tile([P, 2, 512], F32, tag="d")
                for g in range(4):
                    nc.tensor.matmul(
                        m1t[:, g // 2, (g % 2) * JH : (g % 2) * JH + JH],
                        lhsT=w2_s[:, 0, :], rhs=rps[g][:, 0:JH],
                        start=(g % 2 == 0), stop=False, skip_group_check=True,
                    )
                for g in range(4):
                    nc.tensor.matmul(
                        dt[:, g // 2, (g % 2) * JH : (g % 2) * JH + JH],
                        lhsT=w2_s[:, 0, :], rhs=rps[g][:, JH:M],
                        start=(g % 2 == 0), stop=False, skip_group_check=True,
                    )
                for g in range(4):
                    nc.tensor.matmul(
                        dt[:, g // 2, (g % 2) * JH : (g % 2) * JH + JH],
                        lhsT=w2_s[:, 1, :], rhs=rps[g][:, 0:JH],
                        start=False, stop=True, skip_group_check=True,
                    )
                rr_k = rr_pool.tile([P, 2, M], F16)
                nc.scalar.activation(
                    out=rr_k[:, :, :], in_=dt[:, :, 0:M],
                    func=ACTF.Relu, scale=1.0,
                )
                if pend is not None:
                    reduce(pend)
                pend = (m1t, rr_k, k)
            mmid(pend)
            reduce(pend)

            nc.vector.tensor_reduce(
                out=acc[:, :], in_=strip[:, :, :], axis=AX.X, op=ALU.add
            )
            nc.sync.dma_start(out=out[:, :], in_=acc[:, :])

    nc.compile()
    return nc


def _prep_inputs(X, Y, W_xx1, W_yx1, b_xx1, b_yx1, W_xx2, W_yx2):
    """Host-side input prep shared by all cores (except xit)."""
    f = np.float32
    XYT = np.ascontiguousarray(
        np.concatenate([X.transpose(0, 2, 1), Y.transpose(0, 2, 1)], axis=1), f
    )  # [B, 128, N]
    W1ad = np.zeros((P, P), f)
    W1ad[:D, :H] = W_xx1[:D]
    W1ad[D:, H:] = W_yx1[:D]
    W1c = np.ascontiguousarray(np.concatenate([W_xx1[D:], W_yx1[D:]], axis=1), f)
    b1v = np.concatenate([b_xx1, b_yx1]).reshape(P, 1).astype(f)
    W2bd = np.zeros((P, P), f)
    W2bd[:H, :H] = W_xx2
    W2bd[H:, H:] = W_yx2
    return XYT, W1ad, W1c, b1v, W2bd


def _pack_blob(XYT, XiT, W1ad, W1c, b1v):
    """Pack per-core inputs into the [P, BLOB_W] blob (see build_nc)."""
    f = np.float32
    blob = np.zeros((P, BLOB_W), f)
    o = 0
    blob[:, o : o + B * N] = XYT.transpose(1, 0, 2).reshape(P, B * N)
    o += B * N
    blob[:D, o : o + B * NI] = XiT.transpose(1, 0, 2).reshape(D, B * NI)
    blob[D, o : o + B * NI] = 1.0
    o += B * NI
    blob[:, o : o + P] = W1ad
    o += P
    blob[:D, o : o + P] = W1c
    blob[D, o : o + P] = b1v[:, 0]
    o += P
    assert o == BLOB_W
    return blob


def kernel(
    X, Y,
    W_xx1, b_xx1, W_xx2, b_xx2,
    W_xy1, b_xy1, W_xy2, b_xy2,
    W_yx1, b_yx1, W_yx2, b_yx2,
    W_yy1, b_yy1, W_yy2, b_yy2,
    Wd1, bd1, Wd2, bd2,
    _trace=False, _tmpdir=None,
):
    f = np.float32
    X = np.asarray(X, f)
    Y = np.asarray(Y, f)
    XYT, W1ad, W1c, b1v, W2bd = _prep_inputs(
        X, Y, W_xx1, W_yx1, b_xx1, b_yx1, W_xx2, W_yx2
    )
    W2pack = np.concatenate(
        [W2bd, -W2bd, np.eye(P, dtype=np.float32)], axis=1
    ).astype(np.float16)
    W2pack = np.ascontiguousarray(W2pack)

    in_maps = []
    for c in range(NCORES):
        XiT = np.ascontiguousarray(
            X[:, c * NI : (c + 1) * NI, :].transpose(0, 2, 1), f
        )  # [B, 64, NI]
        in_maps.append(
            {
                "blob": _pack_blob(XYT, XiT, W1ad, W1c, b1v),
                "w2pack": W2pack,
            }
        )

    nc = build_nc()
    res = run_bass_kernel_spmd(
        nc,
        in_maps,
        core_ids=list(range(NCORES)),
        trace=_trace,
        tmpdir=_tmpdir,
    )
    acc = np.zeros((P, B), np.float64)
    for r in res.results:
        acc += r["out"].astype(np.float64)
    acc = acc.astype(f)

    # acc[k, b] = sum_i max_j (relu_pre @ W2)[k]  for xx (k<64) / yx (k>=64)
    Zdiff = (acc[:H] - acc[H:]).T + N * (b_xx2 - b_yx2)[None, :]  # [B, H]
    z = np.concatenate([Zdiff, Zdiff], axis=1).astype(f)  # [B, 2H]
    h = np.maximum(z @ Wd1 + bd1, 0.0).astype(f)
    outv = (h @ Wd2 + bd2).astype(f)
    if _trace:
        return outv, res
    return outv


# revision 26
# speedup vs baseline: 1.0697x; 1.0183x over previous
"""Trainium2 Bass kernel for nn_DivergenceRN (gnn_message_passing).

Reference computes, per batch b:
    Z_XX[b,i,:] = max_j relu(X[b,j]@W1a_xx + X[b,i]@W1c_xx + b1_xx) @ W_xx2
    Z_YX[b,i,:] = max_j relu(Y[b,j]@W1a_yx + X[b,i]@W1c_yx + b1_yx) @ W_yx2
    Z = sum_i (Z_XX - Z_YX);  out = relu(cat(Z,Z)@Wd1+bd1)@Wd2+bd2
(The YY / XY branches in the reference are dead code — output-independent.)

Pipeline ("diff-relu max"), partitions = 64 h-channels x {xx, yx};
exploits max(h1,h2) = h1 + relu(h2-h1) per j-half pair so the 384-wide
max over j becomes one engine pass per stage (PSUM has a single DVE
read port, so dual-PSUM tensor ops are illegal and a direct two-stream
max is impossible):
  Phase 1 (once): pa[b] = blockdiag(W1a)^T @ [X^T;Y^T][b] (4 matmuls) ->
     SBUF fp16; pcb[:,u] = [W1c|b1]^T @ [Xi;1] (1 matmul) -> SBUF f32.
  Per 4-unit supergroup (u = (b,i); PSUM: m1-tile bank p = h1(u2p)|h1(u2p+1),
  d-tile bank p = d(u2p)|d(u2p+1)):
    1. relu-pre per unit: rp = max(pa[b] + pcb[:,u], 0) -> fp16; DVE
       tensor_scalar (op0=add per-partition AP, op1=max 0) at 2x, with
       2 of 8 units on ScalarE activation(Relu, bias=AP) for balance.
    2. PE (w2 then -w2, weight-grouped): h1 -> m1-tile; h2 accum -W2@rp1
       -> d-tile (d = h2-h1).  First MM into a bank uses start=True
       (has_written clear is bank-granular!), others start=False.
    3. ScalarE: rr = relu(d-tile) -> SBUF fp16, ONE op per 4 units.
    4. PE identity matmul accumulates rr onto h1 -> m1 = max(h1,h2).
    5. DVE: ONE batched tensor_reduce [P,2,2,192] -> strip[:, b, i:i+4].
  Final: strip summed over i, DMA out; host does cross-core sum + decoder.
Software-pipelined one supergroup deep (steps 4-5 run for group k-1
while PE fills group k).  A custom DVE op (ADD_MAX_REDUCE_K) is kept
registered for reference but is no longer on the hot path.
Sharding: i in [0,384) split across 8 cores (48 rows per core per batch).
"""

import numpy as np

import concourse.bacc as bacc
import concourse.mybir as mybir
import concourse.tile as tile
from concourse.bass_utils import run_bass_kernel_spmd


def _register_addmax():
    """Custom DVE op: out = in0 + in1; accum_out = max over free dim."""
    import concourse.dve_ops as dve_ops
    from concourse.dve_ops import OPS, DveOp
    from concourse.dve_spec import MaxNeg, Spec, Src0, Src1, maxx

    name = "ADD_MAX_REDUCE_K"
    for o in OPS:
        if o.name == name:
            return o

    def _ref(in0, in1, c0, c1, c2):
        b = in0.astype(np.float32) + in1.astype(np.float32)
        return b, b.reshape(b.shape[0], -1).max(axis=-1, keepdims=True)

    op = DveOp(
        name,
        Spec(body=Src0 + Src1, accum=maxx, accum_init=MaxNeg, reference=_ref),
        subdim=False,
        uops_sha={"v3": "b901c41156a86946", "v4": "c9dee8c65593bc95"},
    )
    OPS.append(op)
    dve_ops._SUB_OPCODE_FOR_NAME[name] = (
        dve_ops._CUSTOM_DVE_ROW_BASE + len(OPS) - 1
    )
    return op


ADD_MAX = _register_addmax()

B, N, M, D, H = 4, 384, 384, 64, 64
NCORES = 8
NI = N // NCORES          # i-rows per core per batch
UNITS = B * NI            # 192 (b,i) units per core
JH = M // 2               # 192, half of the j range
P = 2 * H                 # 128 partitions: h x {xx, yx}
BLOB_W = B * N + B * NI + 2 * P   # packed input blob columns

F32 = mybir.dt.float32
F32R = mybir.dt.float32r
F16 = mybir.dt.float16
AX = mybir.AxisListType
ALU = mybir.AluOpType
ACTF = mybir.ActivationFunctionType

# relu-pre engine per (u % 8): D=vector, G=gpsimd, A=scalar
RELU_PAT = "DADDDADD"
G = 4                     # units per group (ACT relu-d batch, PSUM tile)


def build_nc():
    nc = bacc.Bacc("TRN2", target_bir_lowering=False)

    blob = nc.dram_tensor("blob", [P, BLOB_W], F32R, kind="ExternalInput")
    w2pack = nc.dram_tensor("w2pack", [P, 3 * P], F16, kind="ExternalInput")
    out = nc.dram_tensor("out", [P, B], F32, kind="ExternalOutput")

    with tile.TileContext(nc) as tc:
        with (
            tc.tile_pool(name="singles", bufs=1) as singles,
            tc.tile_pool(name="rp", bufs=12) as rp_pool,
            tc.tile_pool(name="rr", bufs=3) as rr_pool,
            tc.tile_pool(name="m1ps", bufs=2, space="PSUM") as m1_pool,
            tc.tile_pool(name="dps", bufs=2, space="PSUM") as d_pool,
        ):
            blob_s = singles.tile([P, BLOB_W], F32R)
            w2_s = singles.tile([P, 3, P], F16)
            pa16 = singles.tile([P, B, N], F16)
            pcb = singles.tile([P, UNITS], F32)
            strip = singles.tile([P, B, NI], F32)
            scrap = singles.tile([P, JH], F16)
            acc = singles.tile([P, B], F32)

            warm = singles.tile([P, 1], F32)
            nc.vector.memset(warm, 0.0)
            nc.scalar.activation(out=warm, in_=warm, func=ACTF.Relu, scale=1.0)

            nc.sync.dma_start(out=blob_s[:, :], in_=blob[:, :])
            nc.sync.dma_start(out=w2_s[:, :, :], in_=w2pack[:, :])
            o = 0
            xyt_s = blob_s[:, o : o + B * N].rearrange("p (b n) -> p b n", b=B)
            o += B * N
            # xitb: [65, UNITS] — rows 0-63 Xi^T, row 64 = 1.0 (bias lane)
            xitb_s = blob_s[0 : D + 1, o : o + B * NI]
            o += B * NI
            w1ad_s = blob_s[:, o : o + P]
            o += P
            # w1cb: [65, 128] — rows 0-63 = [W1c_xx | W1c_yx], row 64 = b1^T
            w1cb_s = blob_s[0 : D + 1, o : o + P]
            o += P
            assert o == BLOB_W

            # Phase 1: pcb (one matmul), pa16 (one matmul + copy per b),
            # using main-loop PSUM tiles (bank regions) to stay in 8 banks.
            ph = [
                m1_pool.tile([P, 2, 512], F32, tag="m", name="phm0"),
                m1_pool.tile([P, 2, 512], F32, tag="m", name="phm1"),
                d_pool.tile([P, 2, 512], F32, tag="d", name="phd0"),
            ]
            nc.tensor.matmul(
                ph[0][:, 0, 0:UNITS], lhsT=w1cb_s, rhs=xitb_s, start=True, stop=True
            )
            nc.vector.tensor_scalar(
                out=pcb[:, :], in0=ph[0][:, 0, 0:UNITS],
                scalar1=0.0, scalar2=None, op0=ALU.add,
            )
            for b in range(B):
                t, r = ph[(b + 1) // 2], (b + 1) % 2
                nc.tensor.matmul(
                    t[:, r, 0:N], lhsT=w1ad_s, rhs=xyt_s[:, b, :],
                    start=True, stop=True,
                )
                nc.scalar.mul(pa16[:, b, :], t[:, r, 0:N], 1.0)

            def relu_pre(u, rp):
                b = u // NI
                kind = RELU_PAT[u % len(RELU_PAT)]
                eng = {"D": nc.vector, "G": nc.gpsimd, "A": None}[kind]
                if eng is None:
                    nc.scalar.activation(
                        out=rp, in_=pa16[:, b, :], func=ACTF.Relu,
                        bias=pcb[:, u : u + 1], scale=1.0,
                    )
                else:
                    eng.tensor_scalar(
                        out=rp, in0=pa16[:, b, :],
                        scalar1=pcb[:, u : u + 1], scalar2=0.0,
                        op0=ALU.add, op1=ALU.max,
                    )

            # Main loop over 4-unit supergroups. PSUM layout per group:
            #   m1-tile [P,2,512]: bank p holds h1(u2p)|h1(u2p+1) (192 each);
            #   d-tile  [P,2,512]: bank p holds d(u2p)|d(u2p+1), d = h2-h1.
            # ScalarE relus the whole d-tile (one op per 4 units); an
            # identity matmul accumulates rr onto h1 so the m1-tile becomes
            # max(h1,h2) pairs; one batched tensor_reduce then maxes all 4
            # units ([P,2,2,192] -> strip[:, b, i:i+4]).
            def mmid(pend):
                m1t, rr, k = pend
                for p in range(2):
                    nc.tensor.matmul(
                        m1t[:, p, 0:M], lhsT=w2_s[:, 2, :], rhs=rr[:, p, :],
                        start=False, stop=True, skip_group_check=True,
                    )

            def reduce(pend):
                m1t, rr, k = pend
                b, i0 = divmod(4 * k, NI)
                nc.vector.tensor_reduce(
                    out=strip[:, b, i0 : i0 + 4],
                    in_=m1t[:, :, 0:M].rearrange("p a (c j) -> p a c j", c=2),
                    axis=AX.X, op=ALU.max,
                )

            pend = None  # (m1_tile, rr, k)
            for k in range(UNITS // 4):
                rps = []
                for g in range(4):
                    rp = rp_pool.tile([P, N], F16)
                    relu_pre(k * 4 + g, rp)
                    rps.append(rp)
                if pend is not None:
                    mmid(pend)
                m1t = m1_pool.tile([P, 2, 512], F32, tag="m")
                dt = d_pool.tile([P, 2, 512], F32, tag="d")
                for g in range(4):
                    nc.tensor.matmul(
                        m1t[:, g // 2, (g % 2) * JH : (g % 2) * JH + JH],
                        lhsT=w2_s[:, 0, :], rhs=rps[g][:, 0:JH],
                        start=(g % 2 == 0), stop=False, skip_group_check=True,
                    )
                for g in range(4):
                    nc.tensor.matmul(
                        dt[:, g // 2, (g % 2) * JH : (g % 2) * JH + JH],
                        lhsT=w2_s[:, 0, :], rhs=rps[g][:, JH:M],
                        start=(g % 2 == 0), stop=False, skip_group_check=True,
                    )
                for g in range(4):
                    nc.tensor.matmul(
                        dt[:, g // 2, (g % 2) * JH : (g % 2) * JH + JH],
                        lhsT=w2_s[:, 1, :], rhs=rps[g][:, 0:JH],
                        start=False, stop=True, skip_group_check=True,
                    )
                rr_k = rr_pool.tile([P, 2, M], F16)
                nc.scalar.activation(
                    out=rr_k[:, :, :], in_=dt[:, :, 0:M],
                    func=ACTF.Relu, scale=1.0,
                )
                if pend is not None:
                    reduce(pend)
                pend = (m1t, rr_k, k)
            mmid(pend)
            reduce(pend)

            nc.vector.tensor_reduce(
                out=acc[:, :], in_=strip[:, :, :], axis=AX.X, op=ALU.add
            )
            nc.sync.dma_start(out=out[:, :], in_=acc[:, :])

    nc.compile()
    return nc


def _prep_inputs(X, Y, W_xx1, W_yx1, b_xx1, b_yx1, W_xx2, W_yx2):
    """Host-side input prep shared by all cores (except xit)."""
    f = np.float32
    XYT = np.ascontiguousarray(
        np.concatenate([X.transpose(0, 2, 1), Y.transpose(0, 2, 1)], axis=1), f
    )  # [B, 128, N]
    W1ad = np.zeros((P, P), f)
    W1ad[:D, :H] = W_xx1[:D]
    W1ad[D:, H:] = W_yx1[:D]
    W1c = np.ascontiguousarray(np.concatenate([W_xx1[D:], W_yx1[D:]], axis=1), f)
    b1v = np.concatenate([b_xx1, b_yx1]).reshape(P, 1).astype(f)
    W2bd = np.zeros((P, P), f)
    W2bd[:H, :H] = W_xx2
    W2bd[H:, H:] = W_yx2
    return XYT, W1ad, W1c, b1v, W2bd


def _pack_blob(XYT, XiT, W1ad, W1c, b1v):
    """Pack per-core inputs into the [P, BLOB_W] blob (see build_nc)."""
    f = np.float32
    blob = np.zeros((P, BLOB_W), f)
    o = 0
    blob[:, o : o + B * N] = XYT.transpose(1, 0, 2).reshape(P, B * N)
    o += B * N
    blob[:D, o : o + B * NI] = XiT.transpose(1, 0, 2).reshape(D, B * NI)
    blob[D, o : o + B * NI] = 1.0
    o += B * NI
    blob[:, o : o + P] = W1ad
    o += P
    blob[:D, o : o + P] = W1c
    blob[D, o : o + P] = b1v[:, 0]
    o += P
    assert o == BLOB_W
    return blob


def kernel(
    X, Y,
    W_xx1, b_xx1, W_xx2, b_xx2,
    W_xy1, b_xy1, W_xy2, b_xy2,
    W_yx1, b_yx1, W_yx2, b_yx2,
    W_yy1, b_yy1, W_yy2, b_yy2,
    Wd1, bd1, Wd2, bd2,
    _trace=False, _tmpdir=None,
):
    f = np.float32
    X = np.asarray(X, f)
    Y = np.asarray(Y, f)
    XYT, W1ad, W1c, b1v, W2bd = _prep_inputs(
        X, Y, W_xx1, W_yx1, b_xx1, b_yx1, W_xx2, W_yx2
    )
    W2pack = np.concatenate(
        [W2bd, -W2bd, np.eye(P, dtype=np.float32)], axis=1
    ).astype(np.float16)
    W2pack = np.ascontiguousarray(W2pack)

    in_maps = []
    for c in range(NCORES):
        XiT = np.ascontiguousarray(
            X[:, c * NI : (c + 1) * NI, :].transpose(0, 2, 1), f
        )  # [B, 64, NI]
        in_maps.append(
            {
                "blob": _pack_blob(XYT, XiT, W1ad, W1c, b1v),
                "w2pack": W2pack,
            }
        )

    nc = build_nc()
    res = run_bass_kernel_spmd(
        nc,
        in_maps,
        core_ids=list(range(NCORES)),
        trace=_trace,
        tmpdir=_tmpdir,
    )
    acc = np.zeros((P, B), np.float64)
    for r in res.results:
        acc += r["out"].astype(np.float64)
    acc = acc.astype(f)

    # acc[k, b] = sum_i max_j (relu_pre @ W2)[k]  for xx (k<64) / yx (k>=64)
    Zdiff = (acc[:H] - acc[H:]).T + N * (b_xx2 - b_yx2)[None, :]  # [B, H]
    z = np.concatenate([Zdiff, Zdiff], axis=1).astype(f)  # [B, 2H]
    h = np.maximum(z @ Wd1 + bd1, 0.0).astype(f)
    outv = (h @ Wd2 + bd2).astype(f)
    if _trace:
        return outv, res
    return outv
